# revision 25
# baseline (speedup 1.0000x reference)
"""Trainium2 Bass kernel for nn_MoELayer (moe_routing).

Strategy (8 cores, SPMD single program):
  out = sum_e combine[:,e] * expert_e(x) with dense per-token top-2 combine
  weights.  Experts 0-3 are "fractal" blocks (rmsnorm -> swiglu(HF=2048) ->
  gamma residual), experts 4-7 plain swiglu (HS=4096).  Every expert's
  swiglu splits additively along its hidden dim, so each core gets a
  uniform pair of jobs over ALL 4096 tokens:
    - half of fractal expert f=c%4 (1024 hidden rows)
    - half of swiglu expert 4+(c%4) (2048 hidden rows)
  fractal = gamma*xn + x + gamma*(swigluA(xn)+swigluB(xn)); the affine part
  (alpha=gamma, beta=1) is carried by half 0 only, via per-core input data
  (alpha vector / beta scalar / one-hot expert selectors).  All cores run
  one identical program; only input values differ.  The router runs
  replicated in fp32 (min top2/top3 logit gap on this data ~1e-4, so bf16
  or f32r routing would flip expert selections).  Expert matmuls run in
  bf16 (PE 1 cyc/row).  gamma is folded into w2 (scaled per D-row before
  the on-chip transpose); alpha*xn and beta*x enter the fractal psum via
  diagonal-matrix matmuls, so each partial-output tile needs exactly one
  eviction op.  Partials are summed across cores with a chunked
  ReduceScatter; the host reassembles the 8 shards.
"""

import os
import sys
import types

sys.path.insert(0, "/opt/trn_rl_repo")

import numpy as np
from contextlib import ExitStack

import concourse.bass as bass
import concourse.tile as tile
from concourse import bacc, mybir
from concourse.bass_utils import run_bass_kernel_spmd
from concourse.masks import make_identity

P = 128
D = 1024
E = 8
HFH = 1024          # fractal half hidden
HSH = 2048          # swiglu half hidden
NCORES = 8
EPS = 1e-6

f32 = mybir.dt.float32
bf16 = mybir.dt.bfloat16
ALU = mybir.AluOpType
ACT = mybir.ActivationFunctionType
AX = mybir.AxisListType

DK = D // P          # 8 D chunks
FI = HFH // P        # 8 fractal hidden chunks
SI = HSH // P        # 16 swiglu hidden chunks


def _install_ntff_hook():
    try:
        from antenv import axon_hooks  # noqa: F401
        return
    except ImportError:
        pass
    try:
        import antenv
        from trn_agent_boot.trn_boot import _ntff_profile_via_ctypes

        mod = types.ModuleType("antenv.axon_hooks")
        hook = _ntff_profile_via_ctypes("/opt/axon/libaxon_pjrt.so")
        mod.get_axon_ntff_profile_hook = lambda: hook
        mod.set_axon_ntff_profile_hook = lambda h: None
        sys.modules["antenv.axon_hooks"] = mod
        antenv.axon_hooks = mod
    except Exception:
        pass


def build(N=4096, TG=256, SC=512):
    NT = N // P          # token tiles
    NG = N // TG         # matmul groups
    TPG = TG // P        # token tiles per group
    NSC = N // SC        # RS chunks
    GPC = SC // TG       # groups per RS chunk
    SH = SC // NCORES    # output rows per core per RS chunk

    nc = bacc.Bacc("TRN2", target_bir_lowering=False, debug=False,
                   num_devices=NCORES)

    # ---- I/O ----
    x_d = nc.dram_tensor("x", [N, D], f32, kind="ExternalInput").ap()
    w1ft_d = nc.dram_tensor("w1ft", [D, HFH], f32, kind="ExternalInput").ap()
    w3ft_d = nc.dram_tensor("w3ft", [D, HFH], f32, kind="ExternalInput").ap()
    w2fr_d = nc.dram_tensor("w2fr", [D, HFH], f32, kind="ExternalInput").ap()
    w1st_d = nc.dram_tensor("w1st", [D, HSH], f32, kind="ExternalInput").ap()
    w3st_d = nc.dram_tensor("w3st", [D, HSH], f32, kind="ExternalInput").ap()
    w2st_d = nc.dram_tensor("w2st", [HSH, D], f32, kind="ExternalInput").ap()
    rwt_d = nc.dram_tensor("rwt", [D, E], f32, kind="ExternalInput").ap()
    rms8_d = nc.dram_tensor("rms8", [P, DK], f32, kind="ExternalInput").ap()
    alp8_d = nc.dram_tensor("alp8", [P, DK], f32, kind="ExternalInput").ap()
    beta1_d = nc.dram_tensor("beta1", [P, 1], f32, kind="ExternalInput").ap()
    gam8_d = nc.dram_tensor("gam8", [P, DK], f32, kind="ExternalInput").ap()
    self8_d = nc.dram_tensor("self8", [P, E], f32, kind="ExternalInput").ap()
    sels8_d = nc.dram_tensor("sels8", [P, E], f32, kind="ExternalInput").ap()
    out_d = nc.dram_tensor("out", [N // NCORES, D], f32,
                           kind="ExternalOutput").ap()

    # ---- internal DRAM (per-group/per-chunk for fine-grained deps) ----
    xT_dram = [nc.dram_tensor(f"xT_i{g}", [TPG, P, D], bf16).ap()
               for g in range(NG)]
    # RS chunk row-ranges (token spans): uniform SC, but the last chunk is
    # split in two so the exposed tail after the final compute is halved.
    chunks = [(s * SC, (s + 1) * SC) for s in range(NSC - 1)]
    chunks += [(N - SC, N - SC // 2), (N - SC // 2, N)]
    rsin = [nc.dram_tensor(f"rsin_i{s}", [b - a, D], f32).ap()
            for s, (a, b) in enumerate(chunks)]
    rsout = [nc.dram_tensor(f"rsout_i{s}", [(b - a) // NCORES, D], f32).ap()
             for s, (a, b) in enumerate(chunks)]

    with tile.TileContext(nc) as tc, ExitStack() as ctx:
        # ---------------- pools ----------------
        const = ctx.enter_context(tc.tile_pool(name="const", bufs=1))
        smalls = ctx.enter_context(tc.tile_pool(name="smalls", bufs=4))
        stg = ctx.enter_context(tc.tile_pool(name="stg", bufs=1))
        xtfp = ctx.enter_context(tc.tile_pool(name="xtfp", bufs=1))
        bfp = ctx.enter_context(tc.tile_pool(name="bfp", bufs=1))
        xtp = ctx.enter_context(tc.tile_pool(name="xtp", bufs=10))
        hp = ctx.enter_context(tc.tile_pool(name="hp", bufs=17))
        silp = ctx.enter_context(tc.tile_pool(name="silp", bufs=3))
        boutp = ctx.enter_context(tc.tile_pool(name="boutp", bufs=3))
        outp = ctx.enter_context(tc.tile_pool(name="outp", bufs=2))
        w1fp = ctx.enter_context(tc.tile_pool(name="w1fp", bufs=1))
        w3fp = ctx.enter_context(tc.tile_pool(name="w3fp", bufs=1))
        w2fp = ctx.enter_context(tc.tile_pool(name="w2fp", bufs=1))
        w1sp = ctx.enter_context(tc.tile_pool(name="w1sp", bufs=1))
        w3sp = ctx.enter_context(tc.tile_pool(name="w3sp", bufs=1))
        w2sp = ctx.enter_context(tc.tile_pool(name="w2sp", bufs=1))
        psA = ctx.enter_context(tc.tile_pool(name="psA", bufs=4, space="PSUM"))
        psB = ctx.enter_context(tc.tile_pool(name="psB", bufs=2, space="PSUM"))

        def ctile(shape, dtype, nm):
            return const.tile(shape, dtype, name=nm, tag=nm)

        # ---------------- constants / small inputs ----------------
        ident_f = ctile([P, P], f32, "identf")
        make_identity(nc, ident_f[:])
        ident_b = ctile([P, P], bf16, "identb")
        make_identity(nc, ident_b[:])

        rwt_sb = ctile([P, DK * E], f32, "rwtsb")
        nc.sync.dma_start(rwt_sb[:].rearrange("p (k e) -> p k e", k=DK),
                          rwt_d.rearrange("(k p) e -> p k e", p=P))
        rms8 = ctile([P, DK], f32, "rms8")
        nc.sync.dma_start(rms8[:], rms8_d[:])
        alp8 = ctile([P, DK], f32, "alp8")
        nc.sync.dma_start(alp8[:], alp8_d[:])
        gam8 = ctile([P, DK], f32, "gam8")
        nc.sync.dma_start(gam8[:], gam8_d[:])
        beta1 = ctile([P, 1], f32, "beta1")
        nc.sync.dma_start(beta1[:], beta1_d[:])
        self8 = ctile([P, E], f32, "self8")
        nc.sync.dma_start(self8[:], self8_d[:])
        sels8 = ctile([P, E], f32, "sels8")
        nc.sync.dma_start(sels8[:], sels8_d[:])

        diag = []
        for k in range(DK):
            dg = ctile([P, P], bf16, f"diag{k}")
            nc.vector.tensor_scalar_mul(dg[:], ident_b[:], alp8[:, k:k + 1])
            diag.append(dg)
        diagb = ctile([P, P], bf16, "diagb")
        nc.vector.tensor_scalar_mul(diagb[:], ident_b[:], beta1[:])

        cf_all = ctile([P, NT], f32, "cfall")
        cs_all = ctile([P, NT], f32, "csall")
        rsq_all = ctile([P, NT], f32, "rsqall")
        ones1 = ctile([1, P], bf16, "ones1")
        nc.vector.memset(ones1[:], 1.0)
        epsb = ctile([P, 1], f32, "epsb")
        nc.vector.memset(epsb[:], EPS)

        # ---------------- weights: gpsimd casting DMA to bf16 (resident) ----
        def load_cast(pool, dram, ncols, n_tiles):
            tiles = []
            for k in range(n_tiles):
                t_bf = pool.tile([P, ncols], bf16, name=f"w{k}")
                nc.gpsimd.dma_start(t_bf[:], dram[k * P:(k + 1) * P, :])
                tiles.append(t_bf)
            return tiles

        w1f = load_cast(w1fp, w1ft_d, HFH, DK)
        w3f = load_cast(w3fp, w3ft_d, HFH, DK)

        # w2f: raw [D, HFH]; scale rows by gamma (per D chunk) then PE
        # transpose into [HFH, D] tiles.
        w2f = [w2fp.tile([P, D], bf16, name=f"w2f{i}") for i in range(FI)]
        for d in range(DK):
            s2 = bfp.tile([P, D], bf16, name="s2")
            nc.gpsimd.dma_start(s2[:], w2fr_d[d * P:(d + 1) * P, :])
            sb = bfp.tile([P, D], bf16, name="sb")
            nc.vector.tensor_scalar_mul(sb[:], s2[:], gam8[:, d:d + 1])
            psw = psA.tile([P, D], bf16, name="ps")
            for i in range(FI):
                nc.tensor.transpose(psw[:, i * P:(i + 1) * P],
                                    sb[:, i * P:(i + 1) * P], ident_b[:])
            for i in range(FI):
                nc.scalar.copy(w2f[i][:, d * P:(d + 1) * P],
                               psw[:, i * P:(i + 1) * P])

        w1s = load_cast(w1sp, w1st_d, HSH, DK)
        w3s = load_cast(w3sp, w3st_d, HSH, DK)
        w2s = load_cast(w2sp, w2st_d, D, SI)

        # ---------------- phase A body (per token tile) ----------------
        def phase_a(t):
            x_f = stg.tile([P, D], f32, name="x_f", tag="x_f", bufs=3)
            nc.sync.dma_start(x_f[:], x_d[t * P:(t + 1) * P, :])

            ssa = smalls.tile([P, 1], f32, name="ssa")
            ssb = smalls.tile([P, 1], f32, name="ssb")
            for half in range(2):
                scr = psA.tile([P, 512], f32, name="ps")
                nc.scalar.activation(scr[:],
                                     x_f[:, half * 512:(half + 1) * 512],
                                     ACT.Square,
                                     accum_out=(ssa if half == 0 else ssb)[:])
            ssum = smalls.tile([P, 1], f32, name="ssum")
            nc.vector.tensor_tensor(ssum[:], ssa[:], ssb[:], op=ALU.add)
            sq = smalls.tile([P, 1], f32, name="sq")
            nc.scalar.activation(sq[:], ssum[:], ACT.Sqrt, bias=epsb[:],
                                 scale=1.0 / D)
            nc.vector.reciprocal(rsq_all[:, t:t + 1], sq[:])

            # fp32 transpose; evict twice: f32 (router) + bf16 (xT)
            xTf = xtfp.tile([P, D], f32, name="xTf")
            xT_t = bfp.tile([P, D], bf16, name="xT_t")
            for half in range(2):
                ps = psA.tile([P, 512], f32, name="ps")
                for j in range(4):
                    k = half * 4 + j
                    nc.tensor.transpose(ps[:, j * P:(j + 1) * P],
                                        x_f[:, k * P:(k + 1) * P], ident_f[:])
                nc.vector.tensor_copy(xTf[:, half * 512:(half + 1) * 512],
                                      ps[:])
                nc.scalar.copy(xT_t[:, half * 512:(half + 1) * 512], ps[:])
            nc.sync.dma_start(xT_dram[t // TPG][t % TPG], xT_t[:])

            # router: logitsT[e, tok] over D chunks, fp32 exact
            pbr = psB.tile([P, D], f32, name="pb")
            for k in range(DK):
                nc.tensor.matmul(pbr[0:E, 0:P], rwt_sb[:, k * E:(k + 1) * E],
                                 xTf[:, k * P:(k + 1) * P],
                                 start=(k == 0), stop=(k == DK - 1))
            lg_sb = smalls.tile([E, P], f32, name="lg_sb", tag="lgsb", bufs=3)
            nc.vector.tensor_copy(lg_sb[:], pbr[0:E, 0:P])
            nc.tensor.matmul(pbr[:, 512:512 + E], lg_sb[:],
                             ident_f[0:E, 0:E], is_transpose=True)
            lg = smalls.tile([P, E], f32, name="lg")
            nc.vector.tensor_copy(lg[:], pbr[:, 512:512 + E])

            # top-2 combine weights (exact in comparisons)
            m1 = smalls.tile([P, 1], f32, name="m1")
            nc.vector.tensor_reduce(m1[:], lg[:], axis=AX.X, op=ALU.max)
            mask1 = smalls.tile([P, E], f32, name="mask1")
            nc.vector.tensor_scalar(mask1[:], lg[:], m1[:], None, op0=ALU.is_ge)
            l2 = smalls.tile([P, E], f32, name="l2")
            nc.vector.scalar_tensor_tensor(l2[:], mask1[:], -1e9, lg[:],
                                           op0=ALU.mult, op1=ALU.add)
            m2 = smalls.tile([P, 1], f32, name="m2")
            nc.vector.tensor_reduce(m2[:], l2[:], axis=AX.X, op=ALU.max)
            negm1 = smalls.tile([P, 1], f32, name="negm1")
            nc.vector.tensor_scalar_mul(negm1[:], m1[:], -1.0)
            p8 = smalls.tile([P, E], f32, name="p8")
            nc.scalar.activation(p8[:], lg[:], ACT.Exp, bias=negm1[:])
            w2v = smalls.tile([P, 1], f32, name="w2v")
            nc.scalar.activation(w2v[:], m2[:], ACT.Exp, bias=negm1[:])
            den = smalls.tile([P, 1], f32, name="den")
            nc.vector.tensor_scalar_add(den[:], w2v[:], 1.0)
            rec = smalls.tile([P, 1], f32, name="rec")
            nc.vector.reciprocal(rec[:], den[:])
            selm = smalls.tile([P, E], f32, name="selm")
            nc.vector.tensor_scalar(selm[:], lg[:], m2[:], None, op0=ALU.is_ge)
            comb = smalls.tile([P, E], f32, name="comb")
            nc.vector.tensor_tensor(comb[:], p8[:], selm[:], op=ALU.mult)
            comb2 = smalls.tile([P, E], f32, name="comb2")
            nc.vector.tensor_scalar_mul(comb2[:], comb[:], rec[:])
            t8 = smalls.tile([P, E], f32, name="t8")
            nc.vector.tensor_tensor(t8[:], comb2[:], self8[:], op=ALU.mult)
            nc.vector.tensor_reduce(cf_all[:, t:t + 1], t8[:], axis=AX.X,
                                    op=ALU.add)
            t8b = smalls.tile([P, E], f32, name="t8b")
            nc.vector.tensor_tensor(t8b[:], comb2[:], sels8[:], op=ALU.mult)
            nc.vector.tensor_reduce(cs_all[:, t:t + 1], t8b[:], axis=AX.X,
                                    op=ALU.add)

        # ---------------- phase A pipelined ahead of B/C groups -------------
        PRO = 2
        for t in range(min(PRO * TPG, NT)):
            phase_a(t)
        for g in range(NG):
            if (g + PRO) * TPG < NT:
                for t in range((g + PRO) * TPG, (g + PRO + 1) * TPG):
                    phase_a(t)
            xtk = []
            for k in range(DK):
                xt_tl = xtp.tile([P, TG], bf16, name="xt_tl")
                nc.sync.dma_start(
                    xt_tl[:].rearrange("p (t c) -> p t c", t=TPG),
                    xT_dram[g][:, :, k * P:(k + 1) * P]
                    .rearrange("t p c -> p t c"))
                xtk.append(xt_tl)

            # broadcast rsq over partitions (K=1 outer product), then
            # xnT_k = xT_k * rms_w * rsq on the fly
            psr = psA.tile([P, TG], f32, name="ps")
            for m in range(TPG):
                tt = g * TPG + m
                rrow = smalls.tile([1, P], bf16, name="rrow")
                psq = psB.tile([P, D], f32, name="pb")
                nc.tensor.matmul(psq[0:1, 0:P], rsq_all[:, tt:tt + 1],
                                 ident_f[:], is_transpose=True)
                nc.vector.tensor_copy(rrow[:], psq[0:1, 0:P])
                nc.tensor.matmul(psr[:, m * P:(m + 1) * P], ones1[:],
                                 rrow[:], start=True, stop=True)
            rsqb = silp.tile([P, TG], bf16, name="rsqb", tag="rsqb", bufs=2)
            nc.vector.tensor_copy(rsqb[:], psr[:])
            xnk = []
            for k in range(DK):
                xn_tl = xtp.tile([P, TG], bf16, name="xn_tl")
                nc.vector.scalar_tensor_tensor(xn_tl[:], xtk[k][:],
                                               rms8[:, k:k + 1], rsqb[:],
                                               op0=ALU.mult, op1=ALU.mult)
                xnk.append(xn_tl)

            # ---- B: fractal half ----
            h1 = []
            for i in range(FI):
                pa = psA.tile([P, TG], f32, name="ps")
                pc = psA.tile([P, TG], f32, name="ps")
                isl = slice(i * P, (i + 1) * P)
                for k in range(DK):
                    nc.tensor.matmul(pa[:], w1f[k][:, isl], xnk[k][:],
                                     start=(k == 0), stop=(k == DK - 1))
                    nc.tensor.matmul(pc[:], w3f[k][:, isl], xnk[k][:],
                                     start=(k == 0), stop=(k == DK - 1))
                sil = silp.tile([P, TG], bf16, name="sil")
                nc.scalar.activation(sil[:], pa[:], ACT.Silu)
                h = hp.tile([P, TG], bf16, name="h")
                nc.vector.tensor_tensor(h[:], sil[:], pc[:], op=ALU.mult)
                h1.append(h)

            bout = []
            for m in range(TPG):
                tt = g * TPG + m
                msl = slice(m * P, (m + 1) * P)
                pb = psB.tile([P, D], f32, name="pb")
                for i in range(FI):
                    nc.tensor.matmul(pb[:, 0:512], h1[i][:, msl],
                                     w2f[i][:, 0:512], start=(i == 0),
                                     stop=False)
                    nc.tensor.matmul(pb[:, 512:1024], h1[i][:, msl],
                                     w2f[i][:, 512:1024], start=(i == 0),
                                     stop=False)
                for k in range(DK):
                    ksl = slice(k * P, (k + 1) * P)
                    nc.tensor.matmul(pb[:, ksl], xnk[k][:, msl], diag[k][:],
                                     start=False, stop=False)
                    nc.tensor.matmul(pb[:, ksl], xtk[k][:, msl], diagb[:],
                                     start=False, stop=(k == 3 or k == DK - 1))
                bo = boutp.tile([P, D], bf16, name="bo")
                nc.vector.tensor_scalar_mul(bo[:], pb[:],
                                            cf_all[:, tt:tt + 1])
                bout.append(bo)

            # ---- C: swiglu half ----
            h2 = []
            for i in range(SI):
                pa = psA.tile([P, TG], f32, name="ps")
                pc = psA.tile([P, TG], f32, name="ps")
                isl = slice(i * P, (i + 1) * P)
                for k in range(DK):
                    nc.tensor.matmul(pa[:], w1s[k][:, isl], xtk[k][:],
                                     start=(k == 0), stop=(k == DK - 1))
                    nc.tensor.matmul(pc[:], w3s[k][:, isl], xtk[k][:],
                                     start=(k == 0), stop=(k == DK - 1))
                sil = silp.tile([P, TG], bf16, name="sil")
                nc.scalar.activation(sil[:], pa[:], ACT.Silu)
                h = hp.tile([P, TG], bf16, name="h")
                nc.vector.tensor_tensor(h[:], sil[:], pc[:], op=ALU.mult)
                h2.append(h)

            for m in range(TPG):
                tt = g * TPG + m
                msl = slice(m * P, (m + 1) * P)
                pb = psB.tile([P, D], f32, name="pb")
                for i in range(SI):
                    nc.tensor.matmul(pb[:, 0:512], h2[i][:, msl],
                                     w2s[i][:, 0:512], start=(i == 0),
                                     stop=(i == SI - 1))
                    nc.tensor.matmul(pb[:, 512:1024], h2[i][:, msl],
                                     w2s[i][:, 512:1024], start=(i == 0),
                                     stop=(i == SI - 1))
                for half in range(2):
                    hs = slice(half * 512, (half + 1) * 512)
                    ot = outp.tile([P, 512], f32, name="ot")
                    nc.vector.scalar_tensor_tensor(
                        ot[:], pb[:, hs], cs_all[:, tt:tt + 1],
                        bout[m][:, hs], op0=ALU.mult, op1=ALU.add)
                    row = tt * P
                    ci = next(i for i, (a, b) in enumerate(chunks)
                              if a <= row < b)
                    rr = row - chunks[ci][0]
                    nc.sync.dma_start(
                        rsin[ci][rr:rr + P, half * 512:(half + 1) * 512],
                        ot[:])

            # ---- ReduceScatter for every chunk completed by this group ----
            done_rows = (g + 1) * TG
            for ci, (a, b) in enumerate(chunks):
                if a < done_rows and b <= done_rows and b > g * TG:
                    nc.gpsimd.collective_compute(
                        "ReduceScatter", ALU.add,
                        replica_groups=[list(range(NCORES))],
                        ins=[rsin[ci][:]],
                        outs=[rsout[ci][:]])
                    sh = (b - a) // NCORES
                    nc.sync.dma_start(
                        out_d[a // NCORES:a // NCORES + sh, :], rsout[ci][:])

    nc.compile()
    return nc


# ---------------------------------------------------------------- host side
_NC_CACHE = {}


def _get_nc(N=4096):
    if N not in _NC_CACHE:
        _install_ntff_hook()
        _NC_CACHE[N] = build(N=N)
    return _NC_CACHE[N]


def make_in_maps(inputs):
    x = np.ascontiguousarray(np.asarray(inputs["x"], np.float32))
    router_w = np.asarray(inputs["router_w"], np.float32)
    frac_rms = np.asarray(inputs["frac_rms"], np.float32)
    frac_w1 = np.asarray(inputs["frac_w1"], np.float32)
    frac_w2 = np.asarray(inputs["frac_w2"], np.float32)
    frac_w3 = np.asarray(inputs["frac_w3"], np.float32)
    frac_gamma = np.asarray(inputs["frac_gamma"], np.float32)
    sw_w1 = np.asarray(inputs["sw_w1"], np.float32)
    sw_w2 = np.asarray(inputs["sw_w2"], np.float32)
    sw_w3 = np.asarray(inputs["sw_w3"], np.float32)

    rwt = np.ascontiguousarray(router_w.T)          # [D, E]
    C = np.ascontiguousarray

    in_maps = []
    for c in range(NCORES):
        f = c % 4
        h = c // 4
        fsl = slice(h * HFH, (h + 1) * HFH)
        ssl = slice(h * HSH, (h + 1) * HSH)
        gam8 = C(frac_gamma[f].reshape(DK, P).T)
        alp8 = gam8 if h == 0 else np.zeros((P, DK), np.float32)
        beta1 = np.full((P, 1), 1.0 if h == 0 else 0.0, np.float32)
        self8 = np.zeros((P, E), np.float32)
        self8[:, f] = 1.0
        sels8 = np.zeros((P, E), np.float32)
        sels8[:, 4 + f] = 1.0
        in_maps.append({
            "x": x,
            "w1ft": C(frac_w1[f, fsl, :].T),
            "w3ft": C(frac_w3[f, fsl, :].T),
            "w2fr": C(frac_w2[f][:, fsl]),
            "w1st": C(sw_w1[f, ssl, :].T),
            "w3st": C(sw_w3[f, ssl, :].T),
            "w2st": C(sw_w2[f][:, ssl].T),
            "rwt": rwt,
            "rms8": C(frac_rms[f].reshape(DK, P).T),
            "alp8": C(alp8),
            "beta1": beta1,
            "gam8": gam8,
            "self8": self8,
            "sels8": sels8,
        })
    return in_maps


def assemble(results, N=4096, SC=512):
    NSC = N // SC
    chunks = [(s * SC, (s + 1) * SC) for s in range(NSC - 1)]
    chunks += [(N - SC, N - SC // 2), (N - SC // 2, N)]
    out = np.empty((N, D), np.float32)
    for c in range(NCORES):
        o = results[c]["out"]          # [N//NCORES, D]
        for a, b in chunks:
            sh = (b - a) // NCORES
            oa = a // NCORES
            out[a + c * sh:a + (c + 1) * sh, :] = o[oa:oa + sh, :]
    return out


def kernel(**inputs):
    N = inputs["x"].shape[0]
    nc = _get_nc(N)
    in_maps = make_in_maps(inputs)
    trace = bool(int(os.environ.get("KERNEL_TRACE", "0")))
    res = run_bass_kernel_spmd(nc, in_maps, list(range(NCORES)), trace=trace)
    kernel.last_exec_ns = res.exec_time_ns
    kernel.last_results = res
    return assemble(res.results, N)


kernel.last_exec_ns = None


# revision 26
# speedup vs baseline: 1.0222x; 1.0222x over previous
"""Trainium2 Bass kernel for nn_MoELayer (moe_routing).

Strategy (8 cores, SPMD single program):
  out = sum_e combine[:,e] * expert_e(x) with dense per-token top-2 combine
  weights.  Experts 0-3 are "fractal" blocks (rmsnorm -> swiglu(HF=2048) ->
  gamma residual), experts 4-7 plain swiglu (HS=4096).  Every expert's
  swiglu splits additively along its hidden dim, so each core gets a
  uniform pair of jobs over ALL 4096 tokens:
    - half of fractal expert f=c%4 (1024 hidden rows)
    - half of swiglu expert 4+(c%4) (2048 hidden rows)
  fractal = gamma*xn + x + gamma*(swigluA(xn)+swigluB(xn)); the affine part
  (alpha=gamma, beta=1) is carried by half 0 only, via per-core input data
  (alpha vector / beta scalar / one-hot expert selectors).  All cores run
  one identical program; only input values differ.  The router runs
  replicated in fp32 (min top2/top3 logit gap on this data ~1e-4, so bf16
  or f32r routing would flip expert selections).  Expert matmuls run in
  bf16 (PE 1 cyc/row).  gamma is folded into w2 (scaled per D-row before
  the on-chip transpose); alpha*xn and beta*x enter the fractal psum via
  diagonal-matrix matmuls, so each partial-output tile needs exactly one
  eviction op.  Partials are summed across cores with a chunked
  ReduceScatter; the host reassembles the 8 shards.
"""

import os
import sys
import types

sys.path.insert(0, "/opt/trn_rl_repo")

import numpy as np
from contextlib import ExitStack

import concourse.bass as bass
import concourse.tile as tile
from concourse import bacc, mybir
from concourse.bass_utils import run_bass_kernel_spmd
from concourse.masks import make_identity

P = 128
D = 1024
E = 8
HFH = 1024          # fractal half hidden
HSH = 2048          # swiglu half hidden
NCORES = 8
EPS = 1e-6

f32 = mybir.dt.float32
bf16 = mybir.dt.bfloat16
ALU = mybir.AluOpType
ACT = mybir.ActivationFunctionType
AX = mybir.AxisListType

DK = D // P          # 8 D chunks
FI = HFH // P        # 8 fractal hidden chunks
SI = HSH // P        # 16 swiglu hidden chunks


def _install_ntff_hook():
    try:
        from antenv import axon_hooks  # noqa: F401
        return
    except ImportError:
        pass
    try:
        import antenv
        from trn_agent_boot.trn_boot import _ntff_profile_via_ctypes

        mod = types.ModuleType("antenv.axon_hooks")
        hook = _ntff_profile_via_ctypes("/opt/axon/libaxon_pjrt.so")
        mod.get_axon_ntff_profile_hook = lambda: hook
        mod.set_axon_ntff_profile_hook = lambda h: None
        sys.modules["antenv.axon_hooks"] = mod
        antenv.axon_hooks = mod
    except Exception:
        pass


def build(N=4096, TG=256, SC=512):
    NT = N // P          # token tiles
    NG = N // TG         # matmul groups
    TPG = TG // P        # token tiles per group
    NSC = N // SC        # RS chunks
    GPC = SC // TG       # groups per RS chunk
    SH = SC // NCORES    # output rows per core per RS chunk

    nc = bacc.Bacc("TRN2", target_bir_lowering=False, debug=False,
                   num_devices=NCORES)

    # ---- I/O ----
    x_d = nc.dram_tensor("x", [N, D], f32, kind="ExternalInput").ap()
    w1ft_d = nc.dram_tensor("w1ft", [D, HFH], f32, kind="ExternalInput").ap()
    w3ft_d = nc.dram_tensor("w3ft", [D, HFH], f32, kind="ExternalInput").ap()
    w2fr_d = nc.dram_tensor("w2fr", [D, HFH], f32, kind="ExternalInput").ap()
    w1st_d = nc.dram_tensor("w1st", [D, HSH], f32, kind="ExternalInput").ap()
    w3st_d = nc.dram_tensor("w3st", [D, HSH], f32, kind="ExternalInput").ap()
    w2st_d = nc.dram_tensor("w2st", [HSH, D], f32, kind="ExternalInput").ap()
    rwt_d = nc.dram_tensor("rwt", [D, E], f32, kind="ExternalInput").ap()
    rms8_d = nc.dram_tensor("rms8", [P, DK], f32, kind="ExternalInput").ap()
    alp8_d = nc.dram_tensor("alp8", [P, DK], f32, kind="ExternalInput").ap()
    beta1_d = nc.dram_tensor("beta1", [P, 1], f32, kind="ExternalInput").ap()
    gam8_d = nc.dram_tensor("gam8", [P, DK], f32, kind="ExternalInput").ap()
    self8_d = nc.dram_tensor("self8", [P, E], f32, kind="ExternalInput").ap()
    sels8_d = nc.dram_tensor("sels8", [P, E], f32, kind="ExternalInput").ap()
    out_d = nc.dram_tensor("out", [N // NCORES, D], f32,
                           kind="ExternalOutput").ap()

    # ---- internal DRAM (per-group/per-chunk for fine-grained deps) ----
    xT_dram = [nc.dram_tensor(f"xT_i{g}", [TPG, P, D], bf16).ap()
               for g in range(NG)]
    xnT_dram = [nc.dram_tensor(f"xnT_i{g}", [TPG, P, D], bf16).ap()
                for g in range(NG)]
    # RS chunk row-ranges (token spans): uniform SC, but the last chunk is
    # split in two so the exposed tail after the final compute is halved.
    chunks = [(s * SC, (s + 1) * SC) for s in range(NSC - 1)]
    chunks += [(N - SC, N - SC // 2), (N - SC // 2, N)]
    rsin = [nc.dram_tensor(f"rsin_i{s}", [b - a, D], f32).ap()
            for s, (a, b) in enumerate(chunks)]
    rsout = [nc.dram_tensor(f"rsout_i{s}", [(b - a) // NCORES, D], f32).ap()
             for s, (a, b) in enumerate(chunks)]

    with tile.TileContext(nc) as tc, ExitStack() as ctx:
        # ---------------- pools ----------------
        const = ctx.enter_context(tc.tile_pool(name="const", bufs=1))
        smalls = ctx.enter_context(tc.tile_pool(name="smalls", bufs=4))
        stg = ctx.enter_context(tc.tile_pool(name="stg", bufs=1))
        xtfp = ctx.enter_context(tc.tile_pool(name="xtfp", bufs=1))
        bfp = ctx.enter_context(tc.tile_pool(name="bfp", bufs=1))
        xtp = ctx.enter_context(tc.tile_pool(name="xtp", bufs=10))
        hp = ctx.enter_context(tc.tile_pool(name="hp", bufs=17))
        silp = ctx.enter_context(tc.tile_pool(name="silp", bufs=3))
        boutp = ctx.enter_context(tc.tile_pool(name="boutp", bufs=3))
        outp = ctx.enter_context(tc.tile_pool(name="outp", bufs=2))
        w1fp = ctx.enter_context(tc.tile_pool(name="w1fp", bufs=1))
        w3fp = ctx.enter_context(tc.tile_pool(name="w3fp", bufs=1))
        w2fp = ctx.enter_context(tc.tile_pool(name="w2fp", bufs=1))
        w1sp = ctx.enter_context(tc.tile_pool(name="w1sp", bufs=1))
        w3sp = ctx.enter_context(tc.tile_pool(name="w3sp", bufs=1))
        w2sp = ctx.enter_context(tc.tile_pool(name="w2sp", bufs=1))
        psA = ctx.enter_context(tc.tile_pool(name="psA", bufs=4, space="PSUM"))
        psB = ctx.enter_context(tc.tile_pool(name="psB", bufs=2, space="PSUM"))

        def ctile(shape, dtype, nm):
            return const.tile(shape, dtype, name=nm, tag=nm)

        # ---------------- constants / small inputs ----------------
        ident_f = ctile([P, P], f32, "identf")
        make_identity(nc, ident_f[:])
        ident_b = ctile([P, P], bf16, "identb")
        make_identity(nc, ident_b[:])

        rwt_sb = ctile([P, DK * E], f32, "rwtsb")
        nc.sync.dma_start(rwt_sb[:].rearrange("p (k e) -> p k e", k=DK),
                          rwt_d.rearrange("(k p) e -> p k e", p=P))
        rms8 = ctile([P, DK], f32, "rms8")
        nc.sync.dma_start(rms8[:], rms8_d[:])
        alp8 = ctile([P, DK], f32, "alp8")
        nc.sync.dma_start(alp8[:], alp8_d[:])
        gam8 = ctile([P, DK], f32, "gam8")
        nc.sync.dma_start(gam8[:], gam8_d[:])
        beta1 = ctile([P, 1], f32, "beta1")
        nc.sync.dma_start(beta1[:], beta1_d[:])
        self8 = ctile([P, E], f32, "self8")
        nc.sync.dma_start(self8[:], self8_d[:])
        sels8 = ctile([P, E], f32, "sels8")
        nc.sync.dma_start(sels8[:], sels8_d[:])

        diag = []
        for k in range(DK):
            dg = ctile([P, P], bf16, f"diag{k}")
            nc.vector.tensor_scalar_mul(dg[:], ident_b[:], alp8[:, k:k + 1])
            diag.append(dg)
        diagb = ctile([P, P], bf16, "diagb")
        nc.vector.tensor_scalar_mul(diagb[:], ident_b[:], beta1[:])

        cf_all = ctile([P, NT], f32, "cfall")
        cs_all = ctile([P, NT], f32, "csall")
        epsb = ctile([P, 1], f32, "epsb")
        nc.vector.memset(epsb[:], EPS)

        # ---------------- weights: gpsimd casting DMA to bf16 (resident) ----
        def load_cast(pool, dram, ncols, n_tiles):
            tiles = []
            for k in range(n_tiles):
                t_bf = pool.tile([P, ncols], bf16, name=f"w{k}")
                nc.gpsimd.dma_start(t_bf[:], dram[k * P:(k + 1) * P, :])
                tiles.append(t_bf)
            return tiles

        w1f = load_cast(w1fp, w1ft_d, HFH, DK)
        w3f = load_cast(w3fp, w3ft_d, HFH, DK)

        # w2f: raw [D, HFH]; scale rows by gamma (per D chunk) then PE
        # transpose into [HFH, D] tiles.
        w2f = [w2fp.tile([P, D], bf16, name=f"w2f{i}") for i in range(FI)]
        for d in range(DK):
            s2 = bfp.tile([P, D], bf16, name="s2")
            nc.gpsimd.dma_start(s2[:], w2fr_d[d * P:(d + 1) * P, :])
            sb = bfp.tile([P, D], bf16, name="sb")
            nc.vector.tensor_scalar_mul(sb[:], s2[:], gam8[:, d:d + 1])
            psw = psA.tile([P, D], bf16, name="ps")
            for i in range(FI):
                nc.tensor.transpose(psw[:, i * P:(i + 1) * P],
                                    sb[:, i * P:(i + 1) * P], ident_b[:])
            for i in range(FI):
                nc.scalar.copy(w2f[i][:, d * P:(d + 1) * P],
                               psw[:, i * P:(i + 1) * P])

        w1s = load_cast(w1sp, w1st_d, HSH, DK)
        w3s = load_cast(w3sp, w3st_d, HSH, DK)
        w2s = load_cast(w2sp, w2st_d, D, SI)

        # ---------------- phase A body (per token tile) ----------------
        def phase_a(t):
            x_f = stg.tile([P, D], f32, name="x_f", tag="x_f", bufs=2)
            nc.sync.dma_start(x_f[:], x_d[t * P:(t + 1) * P, :])

            ssa = smalls.tile([P, 1], f32, name="ssa")
            ssb = smalls.tile([P, 1], f32, name="ssb")
            for half in range(2):
                scr = psA.tile([P, 512], f32, name="ps")
                nc.scalar.activation(scr[:],
                                     x_f[:, half * 512:(half + 1) * 512],
                                     ACT.Square,
                                     accum_out=(ssa if half == 0 else ssb)[:])
            ssum = smalls.tile([P, 1], f32, name="ssum")
            nc.vector.tensor_tensor(ssum[:], ssa[:], ssb[:], op=ALU.add)
            sq = smalls.tile([P, 1], f32, name="sq")
            nc.scalar.activation(sq[:], ssum[:], ACT.Sqrt, bias=epsb[:],
                                 scale=1.0 / D)
            rsq = smalls.tile([P, 1], f32, name="rsq")
            nc.vector.reciprocal(rsq[:], sq[:])

            xn_b = bfp.tile([P, D], bf16, name="xn_b")
            nc.vector.tensor_scalar_mul(xn_b[:], x_f[:], rsq[:])
            x_b = bfp.tile([P, D], bf16, name="x_b")
            nc.scalar.copy(x_b[:], x_f[:])

            # fp32 transpose (router input)
            xTf = xtfp.tile([P, D], f32, name="xTf")
            for half in range(2):
                ps = psA.tile([P, 512], f32, name="ps")
                for j in range(4):
                    k = half * 4 + j
                    nc.tensor.transpose(ps[:, j * P:(j + 1) * P],
                                        x_f[:, k * P:(k + 1) * P], ident_f[:])
                nc.vector.tensor_copy(xTf[:, half * 512:(half + 1) * 512],
                                      ps[:])

            # bf16 transposes -> DRAM
            xnT_t = bfp.tile([P, D], bf16, name="xnT_t")
            xT_t = bfp.tile([P, D], bf16, name="xT_t")
            ps1 = psA.tile([P, D], bf16, name="ps")
            for k in range(DK):
                nc.tensor.transpose(ps1[:, k * P:(k + 1) * P],
                                    xn_b[:, k * P:(k + 1) * P], ident_b[:])
            for k in range(DK):
                nc.vector.tensor_scalar_mul(
                    xnT_t[:, k * P:(k + 1) * P],
                    ps1[:, k * P:(k + 1) * P], rms8[:, k:k + 1])
            ps2 = psA.tile([P, D], bf16, name="ps")
            for k in range(DK):
                nc.tensor.transpose(ps2[:, k * P:(k + 1) * P],
                                    x_b[:, k * P:(k + 1) * P], ident_b[:])
            nc.scalar.copy(xT_t[:], ps2[:])
            nc.sync.dma_start(xnT_dram[t // TPG][t % TPG], xnT_t[:])
            nc.sync.dma_start(xT_dram[t // TPG][t % TPG], xT_t[:])

            # router: logitsT[e, tok] over D chunks, fp32 exact
            pbr = psB.tile([P, D], f32, name="pb")
            for k in range(DK):
                nc.tensor.matmul(pbr[0:E, 0:P], rwt_sb[:, k * E:(k + 1) * E],
                                 xTf[:, k * P:(k + 1) * P],
                                 start=(k == 0), stop=(k == DK - 1))
            lg_sb = smalls.tile([E, P], f32, name="lg_sb", tag="lgsb", bufs=3)
            nc.vector.tensor_copy(lg_sb[:], pbr[0:E, 0:P])
            nc.tensor.matmul(pbr[:, 512:512 + E], lg_sb[:],
                             ident_f[0:E, 0:E], is_transpose=True)
            lg = smalls.tile([P, E], f32, name="lg")
            nc.vector.tensor_copy(lg[:], pbr[:, 512:512 + E])

            # top-2 combine weights (exact in comparisons)
            m1 = smalls.tile([P, 1], f32, name="m1")
            nc.vector.tensor_reduce(m1[:], lg[:], axis=AX.X, op=ALU.max)
            mask1 = smalls.tile([P, E], f32, name="mask1")
            nc.vector.tensor_scalar(mask1[:], lg[:], m1[:], None, op0=ALU.is_ge)
            l2 = smalls.tile([P, E], f32, name="l2")
            nc.vector.scalar_tensor_tensor(l2[:], mask1[:], -1e9, lg[:],
                                           op0=ALU.mult, op1=ALU.add)
            m2 = smalls.tile([P, 1], f32, name="m2")
            nc.vector.tensor_reduce(m2[:], l2[:], axis=AX.X, op=ALU.max)
            negm1 = smalls.tile([P, 1], f32, name="negm1")
            nc.vector.tensor_scalar_mul(negm1[:], m1[:], -1.0)
            p8 = smalls.tile([P, E], f32, name="p8")
            nc.scalar.activation(p8[:], lg[:], ACT.Exp, bias=negm1[:])
            w2v = smalls.tile([P, 1], f32, name="w2v")
            nc.scalar.activation(w2v[:], m2[:], ACT.Exp, bias=negm1[:])
            den = smalls.tile([P, 1], f32, name="den")
            nc.vector.tensor_scalar_add(den[:], w2v[:], 1.0)
            rec = smalls.tile([P, 1], f32, name="rec")
            nc.vector.reciprocal(rec[:], den[:])
            selm = smalls.tile([P, E], f32, name="selm")
            nc.vector.tensor_scalar(selm[:], lg[:], m2[:], None, op0=ALU.is_ge)
            comb = smalls.tile([P, E], f32, name="comb")
            nc.vector.tensor_tensor(comb[:], p8[:], selm[:], op=ALU.mult)
            comb2 = smalls.tile([P, E], f32, name="comb2")
            nc.vector.tensor_scalar_mul(comb2[:], comb[:], rec[:])
            t8 = smalls.tile([P, E], f32, name="t8")
            nc.vector.tensor_tensor(t8[:], comb2[:], self8[:], op=ALU.mult)
            nc.vector.tensor_reduce(cf_all[:, t:t + 1], t8[:], axis=AX.X,
                                    op=ALU.add)
            t8b = smalls.tile([P, E], f32, name="t8b")
            nc.vector.tensor_tensor(t8b[:], comb2[:], sels8[:], op=ALU.mult)
            nc.vector.tensor_reduce(cs_all[:, t:t + 1], t8b[:], axis=AX.X,
                                    op=ALU.add)

        # ---------------- phase A pipelined ahead of B/C groups -------------
        PRO = 2
        for t in range(min(PRO * TPG, NT)):
            phase_a(t)
        for g in range(NG):
            if (g + PRO) * TPG < NT:
                for t in range((g + PRO) * TPG, (g + PRO + 1) * TPG):
                    phase_a(t)
            xnk = []
            xtk = []
            for k in range(DK):
                xn_tl = xtp.tile([P, TG], bf16, name="xn_tl")
                nc.sync.dma_start(
                    xn_tl[:].rearrange("p (t c) -> p t c", t=TPG),
                    xnT_dram[g][:, :, k * P:(k + 1) * P]
                    .rearrange("t p c -> p t c"))
                xnk.append(xn_tl)
                xt_tl = xtp.tile([P, TG], bf16, name="xt_tl")
                nc.sync.dma_start(
                    xt_tl[:].rearrange("p (t c) -> p t c", t=TPG),
                    xT_dram[g][:, :, k * P:(k + 1) * P]
                    .rearrange("t p c -> p t c"))
                xtk.append(xt_tl)

            # ---- B: fractal half ----
            h1 = []
            for i in range(FI):
                pa = psA.tile([P, TG], f32, name="ps")
                pc = psA.tile([P, TG], f32, name="ps")
                isl = slice(i * P, (i + 1) * P)
                for k in range(DK):
                    nc.tensor.matmul(pa[:], w1f[k][:, isl], xnk[k][:],
                                     start=(k == 0), stop=(k == DK - 1))
                    nc.tensor.matmul(pc[:], w3f[k][:, isl], xnk[k][:],
                                     start=(k == 0), stop=(k == DK - 1))
                sil = silp.tile([P, TG], bf16, name="sil")
                nc.scalar.activation(sil[:], pa[:], ACT.Silu)
                h = hp.tile([P, TG], bf16, name="h")
                nc.vector.tensor_tensor(h[:], sil[:], pc[:], op=ALU.mult)
                h1.append(h)

            bout = []
            for m in range(TPG):
                tt = g * TPG + m
                msl = slice(m * P, (m + 1) * P)
                pb = psB.tile([P, D], f32, name="pb")
                for i in range(FI):
                    nc.tensor.matmul(pb[:, 0:512], h1[i][:, msl],
                                     w2f[i][:, 0:512], start=(i == 0),
                                     stop=False)
                    nc.tensor.matmul(pb[:, 512:1024], h1[i][:, msl],
                                     w2f[i][:, 512:1024], start=(i == 0),
                                     stop=False)
                for k in range(DK):
                    ksl = slice(k * P, (k + 1) * P)
                    nc.tensor.matmul(pb[:, ksl], xnk[k][:, msl], diag[k][:],
                                     start=False, stop=False)
                    nc.tensor.matmul(pb[:, ksl], xtk[k][:, msl], diagb[:],
                                     start=False, stop=(k == 3 or k == DK - 1))
                bo = boutp.tile([P, D], bf16, name="bo")
                nc.vector.tensor_scalar_mul(bo[:], pb[:],
                                            cf_all[:, tt:tt + 1])
                bout.append(bo)

            # ---- C: swiglu half ----
            h2 = []
            for i in range(SI):
                pa = psA.tile([P, TG], f32, name="ps")
                pc = psA.tile([P, TG], f32, name="ps")
                isl = slice(i * P, (i + 1) * P)
                for k in range(DK):
                    nc.tensor.matmul(pa[:], w1s[k][:, isl], xtk[k][:],
                                     start=(k == 0), stop=(k == DK - 1))
                    nc.tensor.matmul(pc[:], w3s[k][:, isl], xtk[k][:],
                                     start=(k == 0), stop=(k == DK - 1))
                sil = silp.tile([P, TG], bf16, name="sil")
                nc.scalar.activation(sil[:], pa[:], ACT.Silu)
                h = hp.tile([P, TG], bf16, name="h")
                nc.vector.tensor_tensor(h[:], sil[:], pc[:], op=ALU.mult)
                h2.append(h)

            for m in range(TPG):
                tt = g * TPG + m
                msl = slice(m * P, (m + 1) * P)
                pb = psB.tile([P, D], f32, name="pb")
                for i in range(SI):
                    nc.tensor.matmul(pb[:, 0:512], h2[i][:, msl],
                                     w2s[i][:, 0:512], start=(i == 0),
                                     stop=(i == SI - 1))
                    nc.tensor.matmul(pb[:, 512:1024], h2[i][:, msl],
                                     w2s[i][:, 512:1024], start=(i == 0),
                                     stop=(i == SI - 1))
                for half in range(2):
                    hs = slice(half * 512, (half + 1) * 512)
                    ot = outp.tile([P, 512], f32, name="ot")
                    nc.vector.scalar_tensor_tensor(
                        ot[:], pb[:, hs], cs_all[:, tt:tt + 1],
                        bout[m][:, hs], op0=ALU.mult, op1=ALU.add)
                    row = tt * P
                    ci = next(i for i, (a, b) in enumerate(chunks)
                              if a <= row < b)
                    rr = row - chunks[ci][0]
                    nc.sync.dma_start(
                        rsin[ci][rr:rr + P, half * 512:(half + 1) * 512],
                        ot[:])

            # ---- ReduceScatter for every chunk completed by this group ----
            done_rows = (g + 1) * TG
            for ci, (a, b) in enumerate(chunks):
                if a < done_rows and b <= done_rows and b > g * TG:
                    nc.gpsimd.collective_compute(
                        "ReduceScatter", ALU.add,
                        replica_groups=[list(range(NCORES))],
                        ins=[rsin[ci][:]],
                        outs=[rsout[ci][:]])
                    sh = (b - a) // NCORES
                    nc.sync.dma_start(
                        out_d[a // NCORES:a // NCORES + sh, :], rsout[ci][:])

    nc.compile()
    return nc


# ---------------------------------------------------------------- host side
_NC_CACHE = {}


def _get_nc(N=4096):
    if N not in _NC_CACHE:
        _install_ntff_hook()
        _NC_CACHE[N] = build(N=N)
    return _NC_CACHE[N]


def make_in_maps(inputs):
    x = np.ascontiguousarray(np.asarray(inputs["x"], np.float32))
    router_w = np.asarray(inputs["router_w"], np.float32)
    frac_rms = np.asarray(inputs["frac_rms"], np.float32)
    frac_w1 = np.asarray(inputs["frac_w1"], np.float32)
    frac_w2 = np.asarray(inputs["frac_w2"], np.float32)
    frac_w3 = np.asarray(inputs["frac_w3"], np.float32)
    frac_gamma = np.asarray(inputs["frac_gamma"], np.float32)
    sw_w1 = np.asarray(inputs["sw_w1"], np.float32)
    sw_w2 = np.asarray(inputs["sw_w2"], np.float32)
    sw_w3 = np.asarray(inputs["sw_w3"], np.float32)

    rwt = np.ascontiguousarray(router_w.T)          # [D, E]
    C = np.ascontiguousarray

    in_maps = []
    for c in range(NCORES):
        f = c % 4
        h = c // 4
        fsl = slice(h * HFH, (h + 1) * HFH)
        ssl = slice(h * HSH, (h + 1) * HSH)
        gam8 = C(frac_gamma[f].reshape(DK, P).T)
        alp8 = gam8 if h == 0 else np.zeros((P, DK), np.float32)
        beta1 = np.full((P, 1), 1.0 if h == 0 else 0.0, np.float32)
        self8 = np.zeros((P, E), np.float32)
        self8[:, f] = 1.0
        sels8 = np.zeros((P, E), np.float32)
        sels8[:, 4 + f] = 1.0
        in_maps.append({
            "x": x,
            "w1ft": C(frac_w1[f, fsl, :].T),
            "w3ft": C(frac_w3[f, fsl, :].T),
            "w2fr": C(frac_w2[f][:, fsl]),
            "w1st": C(sw_w1[f, ssl, :].T),
            "w3st": C(sw_w3[f, ssl, :].T),
            "w2st": C(sw_w2[f][:, ssl].T),
            "rwt": rwt,
            "rms8": C(frac_rms[f].reshape(DK, P).T),
            "alp8": C(alp8),
            "beta1": beta1,
            "gam8": gam8,
            "self8": self8,
            "sels8": sels8,
        })
    return in_maps


def assemble(results, N=4096, SC=512):
    NSC = N // SC
    chunks = [(s * SC, (s + 1) * SC) for s in range(NSC - 1)]
    chunks += [(N - SC, N - SC // 2), (N - SC // 2, N)]
    out = np.empty((N, D), np.float32)
    for c in range(NCORES):
        o = results[c]["out"]          # [N//NCORES, D]
        for a, b in chunks:
            sh = (b - a) // NCORES
            oa = a // NCORES
            out[a + c * sh:a + (c + 1) * sh, :] = o[oa:oa + sh, :]
    return out


def kernel(**inputs):
    N = inputs["x"].shape[0]
    nc = _get_nc(N)
    in_maps = make_in_maps(inputs)
    trace = bool(int(os.environ.get("KERNEL_TRACE", "0")))
    res = run_bass_kernel_spmd(nc, in_maps, list(range(NCORES)), trace=trace)
    kernel.last_exec_ns = res.exec_time_ns
    kernel.last_results = res
    return assemble(res.results, N)


kernel.last_exec_ns = None
